# revision 10
# baseline (speedup 1.0000x reference)
"""CrossEntropyLossWithGaussianSmoothedLabels on 8 TRN2 NeuronCores.

Math: the reference's scatter-built smoothed label at class j is exactly
w[|j-t|] for |j-t|<=3 (w = [1, e^-.5, e^-1, e^-2]), clamped writes always
being overwritten by the nearer-distance write. So

  loss = mean_r( W_r * logsumexp(x_r) - sum_o w[|o|] * x_r[t_r+o] )

with W_r = sum of valid window weights. The gather term is computed on the
TensorEngine without any per-row gather:

  sum_r sum_o w[o] x[r, t_r+o] = sum_{|m-n|<=3} (H^T X)[m, n] * w[n-m]

where H is the one-hot target matrix. H^T X is accumulated in PSUM via 6
banded 128x128 fp16 matmuls per 128-row tile (overlapping class blocks with
ownership-deduped band masks). logsumexp runs max-free (|x| < 6) via the
ScalarEngine's fused exp+accumulate. Each core emits per-row W*lse and 6
per-partition band partials; the host sums (the unshard step) in float64.
"""

import math
from contextlib import ExitStack

import numpy as np

import concourse.bacc as bacc
import concourse.tile as tile
from concourse import mybir
from concourse.bass_utils import run_bass_kernel_spmd

P = 128
C = 722
NCORES = 8
ROWS = 16 * 2048
RPC = ROWS // NCORES  # 4096 rows per core
KPM = 4               # row-tiles per macro tile (per DMA)
NB = 6
BLK = [0, 124, 248, 372, 496, 594]  # even bases -> 4B-aligned fp16 slices
URANGES = [(0, 124), (124, 248), (248, 372), (372, 496), (496, 594), (594, 722)]
WDEC = [1.0, math.exp(-0.5), math.exp(-1.0), math.exp(-2.0)]

f32 = mybir.dt.float32
f16 = mybir.dt.float16
i32 = mybir.dt.int32


def _band_masks() -> np.ndarray:
    """[128, 6*128] f32: block-local band weights, each global band entry
    owned by exactly one block (by min(m,n) ownership range)."""
    m = np.zeros((P, NB * P), np.float32)
    for b in range(NB):
        s = BLK[b]
        lo, hi = URANGES[b]
        for i in range(P):
            for o in range(-3, 4):
                j = i + o
                if 0 <= j < P:
                    mg, ng = s + i, s + j
                    if mg < C and ng < C and lo <= min(mg, ng) < hi:
                        m[i, b * P + j] = WDEC[abs(o)]
    return m


def _build(rpc: int, stage: int = 6):
    nt = rpc // P
    nm = nt // KPM
    assert nt % KPM == 0
    nc = bacc.Bacc(
        "TRN2", target_bir_lowering=False, debug=False, num_devices=NCORES
    )
    AF = mybir.ActivationFunctionType
    OP = mybir.AluOpType

    pred = nc.dram_tensor("prediction", [rpc, C], f32, kind="ExternalInput").ap()
    tgt = nc.dram_tensor("target_pair", [rpc, 2], i32, kind="ExternalInput").ap()
    band = nc.dram_tensor("band", [P, NB * P], f32, kind="ExternalInput").ap()
    out = nc.dram_tensor("out", [P, nt + NB], f32, kind="ExternalOutput").ap()

    with tile.TileContext(nc) as tc, ExitStack() as ctx:
        xp = ctx.enter_context(tc.tile_pool(name="x", bufs=8))
        hp = ctx.enter_context(tc.tile_pool(name="h", bufs=4))
        cp = ctx.enter_context(tc.tile_pool(name="xc", bufs=4))
        sp = ctx.enter_context(tc.tile_pool(name="scr", bufs=2))
        sg = ctx.enter_context(tc.tile_pool(name="singles", bufs=1))
        pp = ctx.enter_context(tc.tile_pool(name="psum", bufs=1, space="PSUM"))

        # target first (tiny, gates H/W/PE): contiguous load of int64 pairs
        t_pair = sg.tile([P, nt, 2], i32)
        tgt_v = tgt.rearrange("(i p) two -> p i two", p=P)
        nc.sync.dma_start(out=t_pair[:], in_=tgt_v)

        # prefetch the first row tiles before anything else so ACT can start
        xp_tiles = {}
        nprefetch = 6
        for i in range(min(nprefetch, nt)):
            x = xp.tile([P, C], f32, name=f"x{i}", tag="x")
            nc.sync.dma_start(out=x[:], in_=pred[i * P:(i + 1) * P, :])
            xp_tiles[i] = x

        band_sb = sg.tile([P, NB * P], f32)
        nc.sync.dma_start(out=band_sb[:], in_=band)
        outsb = sg.tile([P, nt + NB], f32)
        nc.vector.memset(outsb[:], 0.0)

        iota_i = sg.tile([P, C], i32)
        if stage >= 4:
            nc.gpsimd.iota(iota_i[:], pattern=[[1, C]], base=0, channel_multiplier=0)
        iota_h = sg.tile([P, C], f16)
        if stage >= 4:
            nc.vector.tensor_copy(out=iota_h[:], in_=iota_i[:])

        # t_f[p, i] = float(target[i*128 + p]) from the low int32 words
        t_f = sg.tile([P, nt], f32)
        if stage >= 3:
            nc.vector.tensor_copy(out=t_f[:], in_=t_pair[:, :, 0])

        sumexp = sg.tile([P, nt], f32)
        wr = sg.tile([P, nt], f32)

        psum_blk = [pp.tile([P, P], f32, name=f"psumblk{b}", tag=f"psum{b}") for b in range(NB)]

        # W_r = 1 + sum_d w_d*([t>=d] + [t<=721-d])
        nc.vector.memset(wr[:], 1.0)
        for d in (1, 2, 3) if stage >= 3 else ():
            tmp = sp.tile([P, nt], f32, tag="wtmp")
            nc.vector.tensor_scalar(
                out=tmp[:], in0=t_f[:], scalar1=d - 0.5, scalar2=WDEC[d],
                op0=OP.is_ge, op1=OP.mult,
            )
            nc.vector.tensor_tensor(out=wr[:], in0=wr[:], in1=tmp[:], op=OP.add)
            tmp2 = sp.tile([P, nt], f32, tag="wtmp")
            nc.vector.tensor_scalar(
                out=tmp2[:], in0=t_f[:], scalar1=(C - 1 - d) + 0.5, scalar2=WDEC[d],
                op0=OP.is_le, op1=OP.mult,
            )
            nc.vector.tensor_tensor(out=wr[:], in0=wr[:], in1=tmp2[:], op=OP.add)

        for i in range(nt) if stage >= 2 else ():
            if i in xp_tiles:
                x = xp_tiles[i]
            else:
                x = xp.tile([P, C], f32, name=f"x{i}", tag="x")
                nc.sync.dma_start(out=x[:], in_=pred[i * P:(i + 1) * P, :])

            xh = cp.tile([P, C], f16, tag="xh")
            if stage >= 5:
                nc.vector.tensor_copy(out=xh[:], in_=x[:])

            esc = sp.tile([P, C], f16, tag="esc")
            nc.scalar.activation(
                out=esc[:], in_=x[:], func=AF.Exp,
                accum_out=sumexp[:, i:i + 1],
            )
            h = hp.tile([P, C], f16)
            if stage >= 4:
                nc.vector.tensor_scalar(
                    out=h[:], in0=iota_h[:], scalar1=t_f[:, i:i + 1],
                    scalar2=None, op0=OP.is_equal, op1=OP.bypass,
                )
            for b in range(NB) if stage >= 5 else ():
                s = BLK[b]
                nc.tensor.matmul(
                    psum_blk[b][:], h[:, s:s + P], xh[:, s:s + P],
                    start=(i == 0), stop=(i == nt - 1),
                )

        lse = sg.tile([P, nt], f32)
        if stage >= 2:
            nc.scalar.activation(out=lse[:], in_=sumexp[:], func=AF.Ln)
        if stage >= 3:
            nc.vector.tensor_tensor(out=outsb[:, 0:nt], in0=wr[:], in1=lse[:], op=OP.mult)
        elif stage >= 2:
            nc.vector.tensor_copy(out=outsb[:, 0:nt], in_=lse[:])
        for b in range(NB) if stage >= 6 else ():
            mscr = sp.tile([P, P], f32, tag="mscr")
            nc.vector.tensor_tensor(
                out=mscr[:], in0=psum_blk[b][:],
                in1=band_sb[:, b * P:(b + 1) * P], op=OP.mult,
            )
            nc.vector.tensor_reduce(
                out=outsb[:, nt + b:nt + b + 1], in_=mscr[:],
                axis=mybir.AxisListType.XYZW, op=OP.add,
            )
        nc.sync.dma_start(out=out, in_=outsb[:])

    nc.compile()
    return nc


def _shard_inputs(prediction: np.ndarray, target: np.ndarray, rpc: int, ncores: int):
    pred = np.ascontiguousarray(np.asarray(prediction, dtype=np.float32)).reshape(-1, C)
    tgt = np.ascontiguousarray(np.asarray(target)).reshape(-1)
    assert tgt.dtype == np.int64
    tgt_pair = tgt.view(np.int32).reshape(-1, 2)  # little-endian: [:, 0] = low word
    band = _band_masks()
    in_maps = []
    for c in range(ncores):
        sl = slice(c * rpc, (c + 1) * rpc)
        in_maps.append({
            "prediction": pred[sl],
            "target_pair": np.ascontiguousarray(tgt_pair[sl]),
            "band": band,
        })
    return in_maps


def _host_combine(results, nt: int) -> np.float32:
    tot = 0.0
    nrows = 0
    for r in results:
        o = np.asarray(r["out"], dtype=np.float64)
        tot += o[:, :nt].sum() - o[:, nt:nt + NB].sum()
        nrows += P * nt
    return np.float32(tot / nrows)


def kernel(prediction: np.ndarray, target: np.ndarray, _trace: bool = False):
    nc = _build(RPC)
    in_maps = _shard_inputs(prediction, target, RPC, NCORES)
    res = run_bass_kernel_spmd(
        nc, in_maps, core_ids=list(range(NCORES)), trace=_trace
    )
    loss = _host_combine(res.results, RPC // P)
    if _trace:
        return loss, res
    return loss


# revision 12
# speedup vs baseline: 1.0943x; 1.0943x over previous
"""CrossEntropyLossWithGaussianSmoothedLabels on 8 TRN2 NeuronCores.

Math: the reference's scatter-built smoothed label at class j is exactly
w[|j-t|] for |j-t|<=3 (w = [1, e^-.5, e^-1, e^-2]), clamped writes always
being overwritten by the nearer-distance write. So

  loss = mean_r( W_r * logsumexp(x_r) - sum_o w[|o|] * x_r[t_r+o] )

with W_r = sum of valid window weights. The gather term is computed on the
TensorEngine without any per-row gather:

  sum_r sum_o w[o] x[r, t_r+o] = sum_{|m-n|<=3} (H^T X)[m, n] * w[n-m]

where H is the one-hot target matrix. H^T X is accumulated in PSUM via 6
banded 128x128 fp16 matmuls per 128-row tile (overlapping class blocks with
ownership-deduped band masks). logsumexp runs max-free (|x| < 6) via the
ScalarEngine's fused exp+accumulate. Each core emits per-row W*lse and 6
per-partition band partials; the host sums (the unshard step) in float64.
"""

import math
from contextlib import ExitStack

import numpy as np

import concourse.bacc as bacc
import concourse.tile as tile
from concourse import mybir
from concourse.bass_utils import run_bass_kernel_spmd

P = 128
C = 722
NCORES = 8
ROWS = 16 * 2048
RPC = ROWS // NCORES  # 4096 rows per core
KPM = 4               # row-tiles per macro tile (per DMA)
NB = 6
BLK = [0, 124, 248, 372, 496, 594]  # even bases -> 4B-aligned fp16 slices
URANGES = [(0, 124), (124, 248), (248, 372), (372, 496), (496, 594), (594, 722)]
WDEC = [1.0, math.exp(-0.5), math.exp(-1.0), math.exp(-2.0)]

f32 = mybir.dt.float32
f16 = mybir.dt.float16
i32 = mybir.dt.int32


def _band_masks() -> np.ndarray:
    """[128, 6*128] f32: block-local band weights, each global band entry
    owned by exactly one block (by min(m,n) ownership range)."""
    m = np.zeros((P, NB * P), np.float32)
    for b in range(NB):
        s = BLK[b]
        lo, hi = URANGES[b]
        for i in range(P):
            for o in range(-3, 4):
                j = i + o
                if 0 <= j < P:
                    mg, ng = s + i, s + j
                    if mg < C and ng < C and lo <= min(mg, ng) < hi:
                        m[i, b * P + j] = WDEC[abs(o)]
    return m


def _build(rpc: int, stage: int = 6):
    nt = rpc // P
    nm = nt // KPM
    assert nt % KPM == 0
    nc = bacc.Bacc(
        "TRN2", target_bir_lowering=False, debug=False, num_devices=NCORES
    )
    AF = mybir.ActivationFunctionType
    OP = mybir.AluOpType

    pred = nc.dram_tensor("prediction", [rpc, C], f32, kind="ExternalInput").ap()
    tgt = nc.dram_tensor("target_pair", [rpc, 2], i32, kind="ExternalInput").ap()
    band = nc.dram_tensor("band", [P, NB * P], f32, kind="ExternalInput").ap()
    out = nc.dram_tensor("out", [P, nt + NB], f32, kind="ExternalOutput").ap()

    with tile.TileContext(nc) as tc, ExitStack() as ctx:
        xp = ctx.enter_context(tc.tile_pool(name="x", bufs=8))
        hp = ctx.enter_context(tc.tile_pool(name="h", bufs=4))
        cp = ctx.enter_context(tc.tile_pool(name="xc", bufs=4))
        sp = ctx.enter_context(tc.tile_pool(name="scr", bufs=2))
        sg = ctx.enter_context(tc.tile_pool(name="singles", bufs=1))
        pp = ctx.enter_context(tc.tile_pool(name="psum", bufs=1, space="PSUM"))

        # row r lives at partition r // nt, column r % nt: every DMA line is
        # per-partition contiguous (target: 64 i32; prediction: 2888B rows)
        t_pair = sg.tile([P, nt, 2], i32)
        tgt_v = tgt.rearrange("(p i) two -> p i two", i=nt)
        nc.sync.dma_start(out=t_pair[:], in_=tgt_v)
        pred_v = pred.rearrange("(p i) c -> p i c", i=nt)

        # prefetch the first row tiles before anything else so ACT can start
        xp_tiles = {}
        nprefetch = 6
        for i in range(min(nprefetch, nt)):
            x = xp.tile([P, C], f32, name=f"x{i}", tag="x")
            nc.sync.dma_start(out=x[:], in_=pred_v[:, i, :])
            xp_tiles[i] = x

        band_sb = sg.tile([P, NB * P], f32)
        nc.sync.dma_start(out=band_sb[:], in_=band)
        outsb = sg.tile([P, nt + NB], f32)
        nc.vector.memset(outsb[:], 0.0)

        iota_i = sg.tile([P, C], i32)
        if stage >= 4:
            nc.gpsimd.iota(iota_i[:], pattern=[[1, C]], base=0, channel_multiplier=0)
        iota_h = sg.tile([P, C], f16)
        if stage >= 4:
            nc.vector.tensor_copy(out=iota_h[:], in_=iota_i[:])

        # t_f[p, i] = float(target[i*128 + p]) from the low int32 words
        t_f = sg.tile([P, nt], f32)
        if stage >= 3:
            nc.vector.tensor_copy(out=t_f[:], in_=t_pair[:, :, 0])

        sumexp = sg.tile([P, nt], f32)
        wr = sg.tile([P, nt], f32)

        psum_blk = [pp.tile([P, P], f32, name=f"psumblk{b}", tag=f"psum{b}") for b in range(NB)]

        # W_r = 1 + sum_d w_d*([t>=d] + [t<=721-d])
        nc.vector.memset(wr[:], 1.0)
        for d in (1, 2, 3) if stage >= 3 else ():
            tmp = sp.tile([P, nt], f32, tag="wtmp")
            nc.vector.tensor_scalar(
                out=tmp[:], in0=t_f[:], scalar1=d - 0.5, scalar2=WDEC[d],
                op0=OP.is_ge, op1=OP.mult,
            )
            nc.vector.tensor_tensor(out=wr[:], in0=wr[:], in1=tmp[:], op=OP.add)
            tmp2 = sp.tile([P, nt], f32, tag="wtmp")
            nc.vector.tensor_scalar(
                out=tmp2[:], in0=t_f[:], scalar1=(C - 1 - d) + 0.5, scalar2=WDEC[d],
                op0=OP.is_le, op1=OP.mult,
            )
            nc.vector.tensor_tensor(out=wr[:], in0=wr[:], in1=tmp2[:], op=OP.add)

        for i in range(nt) if stage >= 2 else ():
            if i in xp_tiles:
                x = xp_tiles[i]
            else:
                x = xp.tile([P, C], f32, name=f"x{i}", tag="x")
                nc.sync.dma_start(out=x[:], in_=pred_v[:, i, :])

            xh = cp.tile([P, C], f16, tag="xh")
            if stage >= 5:
                nc.vector.tensor_copy(out=xh[:], in_=x[:])

            esc = sp.tile([P, C], f16, tag="esc")
            nc.scalar.activation(
                out=esc[:], in_=x[:], func=AF.Exp,
                accum_out=sumexp[:, i:i + 1],
            )
            h = hp.tile([P, C], f16)
            if stage >= 4:
                nc.vector.tensor_scalar(
                    out=h[:], in0=iota_h[:], scalar1=t_f[:, i:i + 1],
                    scalar2=None, op0=OP.is_equal, op1=OP.bypass,
                )
            for b in range(NB) if stage >= 5 else ():
                s = BLK[b]
                nc.tensor.matmul(
                    psum_blk[b][:], h[:, s:s + P], xh[:, s:s + P],
                    start=(i == 0), stop=(i == nt - 1),
                )

        lse = sg.tile([P, nt], f32)
        if stage >= 2:
            nc.scalar.activation(out=lse[:], in_=sumexp[:], func=AF.Ln)
        if stage >= 3:
            nc.vector.tensor_tensor(out=outsb[:, 0:nt], in0=wr[:], in1=lse[:], op=OP.mult)
        elif stage >= 2:
            nc.vector.tensor_copy(out=outsb[:, 0:nt], in_=lse[:])
        for b in range(NB) if stage >= 6 else ():
            mscr = sp.tile([P, P], f32, tag="mscr")
            nc.vector.tensor_tensor(
                out=mscr[:], in0=psum_blk[b][:],
                in1=band_sb[:, b * P:(b + 1) * P], op=OP.mult,
            )
            nc.vector.tensor_reduce(
                out=outsb[:, nt + b:nt + b + 1], in_=mscr[:],
                axis=mybir.AxisListType.X, op=OP.add,
            )
        nc.sync.dma_start(out=out, in_=outsb[:])

    nc.compile()
    return nc


def _shard_inputs(prediction: np.ndarray, target: np.ndarray, rpc: int, ncores: int):
    pred = np.ascontiguousarray(np.asarray(prediction, dtype=np.float32)).reshape(-1, C)
    tgt = np.ascontiguousarray(np.asarray(target)).reshape(-1)
    assert tgt.dtype == np.int64
    tgt_pair = tgt.view(np.int32).reshape(-1, 2)  # little-endian: [:, 0] = low word
    band = _band_masks()
    in_maps = []
    for c in range(ncores):
        sl = slice(c * rpc, (c + 1) * rpc)
        in_maps.append({
            "prediction": pred[sl],
            "target_pair": np.ascontiguousarray(tgt_pair[sl]),
            "band": band,
        })
    return in_maps


def _host_combine(results, nt: int) -> np.float32:
    tot = 0.0
    nrows = 0
    for r in results:
        o = np.asarray(r["out"], dtype=np.float64)
        tot += o[:, :nt].sum() - o[:, nt:nt + NB].sum()
        nrows += P * nt
    return np.float32(tot / nrows)


def kernel(prediction: np.ndarray, target: np.ndarray, _trace: bool = False):
    nc = _build(RPC)
    in_maps = _shard_inputs(prediction, target, RPC, NCORES)
    res = run_bass_kernel_spmd(
        nc, in_maps, core_ids=list(range(NCORES)), trace=_trace
    )
    loss = _host_combine(res.results, RPC // P)
    if _trace:
        return loss, res
    return loss


# revision 13
# speedup vs baseline: 1.1710x; 1.0701x over previous
"""CrossEntropyLossWithGaussianSmoothedLabels on 8 TRN2 NeuronCores.

Math: the reference's scatter-built smoothed label at class j is exactly
w[|j-t|] for |j-t|<=3 (w = [1, e^-.5, e^-1, e^-2]), clamped writes always
being overwritten by the nearer-distance write. So

  loss = mean_r( W_r * logsumexp(x_r) - sum_o w[|o|] * x_r[t_r+o] )

with W_r = sum of valid window weights. The gather term is computed on the
TensorEngine without any per-row gather:

  sum_r sum_o w[o] x[r, t_r+o] = sum_{|m-n|<=3} (H^T X)[m, n] * w[n-m]

where H is the one-hot target matrix. H^T X is accumulated in PSUM via 6
banded 128x128 fp16 matmuls per 128-row tile (overlapping class blocks with
ownership-deduped band masks). logsumexp runs max-free (|x| < 6) via the
ScalarEngine's fused exp+accumulate. Each core emits per-row W*lse and 6
per-partition band partials; the host sums (the unshard step) in float64.
"""

import math
from contextlib import ExitStack

import numpy as np

import concourse.bacc as bacc
import concourse.tile as tile
from concourse import mybir
from concourse.bass_utils import run_bass_kernel_spmd

P = 128
C = 722
NCORES = 8
ROWS = 16 * 2048
RPC = ROWS // NCORES  # 4096 rows per core
KPM = 4               # row-tiles per macro tile (per DMA)
NB = 6
BLK = [0, 124, 248, 372, 496, 594]  # even bases -> 4B-aligned fp16 slices
URANGES = [(0, 124), (124, 248), (248, 372), (372, 496), (496, 594), (594, 722)]
WDEC = [1.0, math.exp(-0.5), math.exp(-1.0), math.exp(-2.0)]

f32 = mybir.dt.float32
f16 = mybir.dt.float16
i32 = mybir.dt.int32


def _band_masks() -> np.ndarray:
    """[128, 6*128] f32: block-local band weights, each global band entry
    owned by exactly one block (by min(m,n) ownership range)."""
    m = np.zeros((P, NB * P), np.float32)
    for b in range(NB):
        s = BLK[b]
        lo, hi = URANGES[b]
        for i in range(P):
            for o in range(-3, 4):
                j = i + o
                if 0 <= j < P:
                    mg, ng = s + i, s + j
                    if mg < C and ng < C and lo <= min(mg, ng) < hi:
                        m[i, b * P + j] = WDEC[abs(o)]
    return m


def _build(rpc: int, stage: int = 6):
    nt = rpc // P
    nm = nt // KPM
    assert nt % KPM == 0
    nc = bacc.Bacc(
        "TRN2", target_bir_lowering=False, debug=False, num_devices=NCORES
    )
    AF = mybir.ActivationFunctionType
    OP = mybir.AluOpType

    pred = nc.dram_tensor("prediction", [rpc, C], f32, kind="ExternalInput").ap()
    tgt = nc.dram_tensor("target_pair", [rpc, 2], i32, kind="ExternalInput").ap()
    band = nc.dram_tensor("band", [P, NB * P], f32, kind="ExternalInput").ap()
    out = nc.dram_tensor("out", [P, nt + NB], f32, kind="ExternalOutput").ap()

    with tile.TileContext(nc) as tc, ExitStack() as ctx:
        xp = ctx.enter_context(tc.tile_pool(name="x", bufs=12))
        hp = ctx.enter_context(tc.tile_pool(name="h", bufs=4))
        cp = ctx.enter_context(tc.tile_pool(name="xc", bufs=4))
        sp = ctx.enter_context(tc.tile_pool(name="scr", bufs=2))
        sg = ctx.enter_context(tc.tile_pool(name="singles", bufs=1))
        pp = ctx.enter_context(tc.tile_pool(name="psum", bufs=1, space="PSUM"))

        # warm the Exp/Ln ACT table set at t~0 so the first real exp
        # doesn't eat the ~2.6us table load
        warm = sg.tile([P, 1], f32)
        nc.vector.memset(warm[:], 0.0)
        nc.scalar.activation(out=warm[:], in_=warm[:], func=AF.Exp)

        # row r lives at partition r // nt, column r % nt: every DMA line is
        # per-partition contiguous (target: 64 i32; prediction: 2888B rows)
        t_pair = sg.tile([P, nt, 2], i32)
        tgt_v = tgt.rearrange("(p i) two -> p i two", i=nt)
        nc.sync.dma_start(out=t_pair[:], in_=tgt_v)
        pred_v = pred.rearrange("(p i) c -> p i c", i=nt)

        # prefetch the first row tiles before anything else so ACT can start
        xp_tiles = {}
        nprefetch = 6
        for i in range(min(nprefetch, nt)):
            x = xp.tile([P, C], f32, name=f"x{i}", tag="x")
            eng = nc.sync if i % 2 == 0 else nc.gpsimd
            eng.dma_start(out=x[:], in_=pred_v[:, i, :])
            xp_tiles[i] = x

        band_sb = sg.tile([P, NB * P], f32)
        nc.sync.dma_start(out=band_sb[:], in_=band)
        outsb = sg.tile([P, nt + NB], f32)
        nc.vector.memset(outsb[:], 0.0)

        iota_i = sg.tile([P, C], i32)
        if stage >= 4:
            nc.gpsimd.iota(iota_i[:], pattern=[[1, C]], base=0, channel_multiplier=0)
        iota_h = sg.tile([P, C], f16)
        if stage >= 4:
            nc.vector.tensor_copy(out=iota_h[:], in_=iota_i[:])

        # t_f[p, i] = float(target[i*128 + p]) from the low int32 words
        t_f = sg.tile([P, nt], f32)
        if stage >= 3:
            nc.vector.tensor_copy(out=t_f[:], in_=t_pair[:, :, 0])

        sumexp = sg.tile([P, nt], f32)
        wr = sg.tile([P, nt], f32)

        psum_blk = [pp.tile([P, P], f32, name=f"psumblk{b}", tag=f"psum{b}") for b in range(NB)]

        # W_r = 1 + sum_d w_d*([t>=d] + [t<=721-d])
        nc.vector.memset(wr[:], 1.0)
        for d in (1, 2, 3) if stage >= 3 else ():
            tmp = sp.tile([P, nt], f32, tag="wtmp")
            nc.vector.tensor_scalar(
                out=tmp[:], in0=t_f[:], scalar1=d - 0.5, scalar2=WDEC[d],
                op0=OP.is_ge, op1=OP.mult,
            )
            nc.vector.tensor_tensor(out=wr[:], in0=wr[:], in1=tmp[:], op=OP.add)
            tmp2 = sp.tile([P, nt], f32, tag="wtmp")
            nc.vector.tensor_scalar(
                out=tmp2[:], in0=t_f[:], scalar1=(C - 1 - d) + 0.5, scalar2=WDEC[d],
                op0=OP.is_le, op1=OP.mult,
            )
            nc.vector.tensor_tensor(out=wr[:], in0=wr[:], in1=tmp2[:], op=OP.add)

        for i in range(nt) if stage >= 2 else ():
            if i in xp_tiles:
                x = xp_tiles[i]
            else:
                x = xp.tile([P, C], f32, name=f"x{i}", tag="x")
                eng = nc.sync if i % 2 == 0 else nc.gpsimd
                eng.dma_start(out=x[:], in_=pred_v[:, i, :])

            xh = cp.tile([P, C], f16, tag="xh")
            if stage >= 5:
                nc.vector.tensor_copy(out=xh[:], in_=x[:])

            esc = sp.tile([P, C], f16, tag="esc")
            nc.scalar.activation(
                out=esc[:], in_=x[:], func=AF.Exp,
                accum_out=sumexp[:, i:i + 1],
            )
            h = hp.tile([P, C], f16)
            if stage >= 4:
                nc.vector.tensor_scalar(
                    out=h[:], in0=iota_h[:], scalar1=t_f[:, i:i + 1],
                    scalar2=None, op0=OP.is_equal, op1=OP.bypass,
                )
            for b in range(NB) if stage >= 5 else ():
                s = BLK[b]
                nc.tensor.matmul(
                    psum_blk[b][:], h[:, s:s + P], xh[:, s:s + P],
                    start=(i == 0), stop=(i == nt - 1),
                )

        lse = sg.tile([P, nt], f32)
        if stage >= 2:
            nc.scalar.activation(out=lse[:], in_=sumexp[:], func=AF.Ln)
        if stage >= 3:
            nc.vector.tensor_tensor(out=outsb[:, 0:nt], in0=wr[:], in1=lse[:], op=OP.mult)
        elif stage >= 2:
            nc.vector.tensor_copy(out=outsb[:, 0:nt], in_=lse[:])
        for b in range(NB) if stage >= 6 else ():
            mscr = sp.tile([P, P], f32, tag="mscr")
            nc.vector.tensor_tensor(
                out=mscr[:], in0=psum_blk[b][:],
                in1=band_sb[:, b * P:(b + 1) * P], op=OP.mult,
            )
            nc.vector.tensor_reduce(
                out=outsb[:, nt + b:nt + b + 1], in_=mscr[:],
                axis=mybir.AxisListType.X, op=OP.add,
            )
        nc.sync.dma_start(out=out, in_=outsb[:])

    nc.compile()
    return nc


def _shard_inputs(prediction: np.ndarray, target: np.ndarray, rpc: int, ncores: int):
    pred = np.ascontiguousarray(np.asarray(prediction, dtype=np.float32)).reshape(-1, C)
    tgt = np.ascontiguousarray(np.asarray(target)).reshape(-1)
    assert tgt.dtype == np.int64
    tgt_pair = tgt.view(np.int32).reshape(-1, 2)  # little-endian: [:, 0] = low word
    band = _band_masks()
    in_maps = []
    for c in range(ncores):
        sl = slice(c * rpc, (c + 1) * rpc)
        in_maps.append({
            "prediction": pred[sl],
            "target_pair": np.ascontiguousarray(tgt_pair[sl]),
            "band": band,
        })
    return in_maps


def _host_combine(results, nt: int) -> np.float32:
    tot = 0.0
    nrows = 0
    for r in results:
        o = np.asarray(r["out"], dtype=np.float64)
        tot += o[:, :nt].sum() - o[:, nt:nt + NB].sum()
        nrows += P * nt
    return np.float32(tot / nrows)


def kernel(prediction: np.ndarray, target: np.ndarray, _trace: bool = False):
    nc = _build(RPC)
    in_maps = _shard_inputs(prediction, target, RPC, NCORES)
    res = run_bass_kernel_spmd(
        nc, in_maps, core_ids=list(range(NCORES)), trace=_trace
    )
    loss = _host_combine(res.results, RPC // P)
    if _trace:
        return loss, res
    return loss
